# revision 1
# baseline (speedup 1.0000x reference)
"""Trainium2 Bass kernel for StyleGAN2-style upsampling ConvLayer.

Reference computation (per image):
  y = conv_transpose2d(x, (w*WSCALE), stride=2)      # 512ch 64x64 -> 256ch 129x129
  y = upfirdn2d(y, fir([1,3,3,1]), pad=1, gain=4)    # 4x4 blur   -> 128x128
  y = clamp(lrelu(y + bias, 0.2) * sqrt(2), +-256)

Hybrid factorization (validated exact vs reference):
  - Fold the *horizontal* FIR into the weights:
      W_h[o,i,a,u] = (WSCALE*4/64) * sum_b w[o,i,a,b] * [1,3,3,1][u-b]   (3x6 taps)
  - PE computes the vertically-sparse intermediate zz (129 rows x 128 cols):
      zz[2P+s, 2Q+t] = sum_{ic,da,du} W_h[s+2da, t+2du] * x[ic, P-da, Q+1-du]
    as fp32r channel-contraction matmuls (24 or 12 per PSUM group of N=512).
  - DVE applies the vertical FIR [1,3,3,1] as three 2-tap box passes
    (binomial factorization), pure tensor_tensor adds.
  - ACT evacuates PSUM (column-interleaving parities) and applies the
    Prelu epilogue; DVE clamps; contiguous DMA out.

Sharding: data parallel, 2 images per core across 8 NeuronCores.
"""

import numpy as np

N_CORES = 8
IMG_PER_CORE = 2
IN_CH, OUT_CH, K, UP = 512, 256, 3, 2
H = W = 64
WSCALE = float(1.0 / np.sqrt(K * K * IN_CH))
ACT_GAIN = float(np.sqrt(2.0))
CLAMP = 256.0
ALPHA = 0.2
R = 8                  # parity rows per PE group -> matmul N = R*64 = 512
N_RB = H // R          # 8 full zz tiles (16 rows each) + 1 tail tile (2 rows)
N_ICC = IN_CH // 128   # 4 ic chunks
N_OCC = OUT_CH // 128  # 2 oc chunks

_CACHE = {}


def _prep_wh(weight: np.ndarray) -> np.ndarray:
    """wh[occhunk, ic, icchunk, a(3), u(6), oc] float32 with all FIR scales folded."""
    fir4 = np.array([1.0, 3.0, 3.0, 1.0], np.float64)
    w64 = weight.astype(np.float64) * (WSCALE * 4.0 / 64.0)
    W_h = np.zeros((OUT_CH, IN_CH, 3, 6), np.float64)
    for b in range(3):
        W_h[:, :, :, b:b + 4] += w64[:, :, :, b:b + 1] * fir4[None, None, None]
    arr = W_h.reshape(N_OCC, 128, N_ICC, 128, 3, 6)  # [oa, o, c, i, a, u]
    wh = np.ascontiguousarray(
        arr.transpose(0, 3, 2, 4, 5, 1).astype(np.float32))  # [oa, i, c, a, u, o]
    return wh


def _build_nc(n_img: int, n_occ: int, n_img_store: int | None = None):
    # n_img_store < n_img makes later images overwrite earlier output rows —
    # used only by timing harnesses to scale compute at fixed I/O shapes.
    if n_img_store is None:
        n_img_store = n_img
    import concourse.bacc as bacc
    import concourse.mybir as mybir
    import concourse.tile as tile

    f32 = mybir.dt.float32
    f32r = mybir.dt.float32r
    Prelu = mybir.ActivationFunctionType.Prelu
    Copy = mybir.ActivationFunctionType.Copy
    AluOp = mybir.AluOpType

    nc = bacc.Bacc()
    xp_ext = nc.declare_dram_parameter(
        "xp", [n_img, N_ICC, 128, H + 2, W + 2], f32, isOutput=False)
    wh_ext = nc.declare_dram_parameter(
        "wh", [N_OCC, 128, N_ICC, 3, 6, 128], f32, isOutput=False)
    bg_ext = nc.declare_dram_parameter("bg", [128, N_OCC], f32, isOutput=False)
    out_ext = nc.declare_dram_parameter(
        "out", [n_img_store, OUT_CH, 2 * H, 2 * W], f32, isOutput=True)

    with tile.TileContext(nc) as tc:
        with (
            tc.tile_pool(name="wpool", bufs=1) as wpool,
            tc.tile_pool(name="xpool", bufs=2) as xpool,
            tc.tile_pool(name="zpool", bufs=10) as zpool,
            tc.tile_pool(name="tpool", bufs=2) as tpool,
            tc.tile_pool(name="ypool", bufs=2) as ypool,
            tc.tile_pool(name="cpool", bufs=1) as cpool,
            tc.tile_pool(name="ppool", bufs=8, space="PSUM") as ppool,
        ):
            bt = cpool.tile([128, N_OCC], f32)
            nc.sync.dma_start(out=bt[:], in_=bg_ext[:])
            zrow = cpool.tile([128, 1, 2 * W], f32)  # zero boundary row
            nc.vector.memset(zrow[:], 0.0)

            for oa in range(n_occ):
                wt = wpool.tile([128, N_ICC * 3 * 6 * 128], f32r, tag="wt")
                nc.sync.dma_start(out=wt[:], in_=wh_ext[oa].bitcast(f32r))
                for img in range(n_img):
                    zz = [None] * (N_RB + 1)

                    def pe_tile(rb):
                        # x tiles: padded rows [8rb, 8rb+9) (or [64,66) for rb=8)
                        nrow = 9 if rb < N_RB else 2
                        r0 = rb * R
                        xts = []
                        for c in range(N_ICC):
                            xt = xpool.tile([128, nrow, W + 2], f32r, tag=f"x{c}")
                            nc.sync.dma_start(
                                out=xt[:],
                                in_=xp_ext[img, c, :, r0:r0 + nrow, :].bitcast(f32r))
                            xts.append(xt)
                        if rb < N_RB:
                            zt = zpool.tile([128, 2 * R, 2 * W], f32, tag="zz")
                        else:
                            zt = zpool.tile([128, 2, 2 * W], f32, tag="zz")
                            nc.vector.memset(zt[:], 0.0)
                        zz[rb] = zt
                        rows = R if rb < N_RB else 1
                        s_list = (0, 1) if rb < N_RB else (0,)
                        for s in s_list:
                            da_list = (0, 1) if s == 0 else (0,)
                            for t in range(2):
                                ps = ppool.tile([128, rows * W], f32, tag="ps")
                                nmm = len(da_list) * 3 * N_ICC
                                j = 0
                                for c in range(N_ICC):
                                    for da in da_list:
                                        a = s + 2 * da
                                        for du in range(3):
                                            idx = (c * 3 + a) * 6 + (t + 2 * du)
                                            rhs = xts[c][:, 1 - da:1 - da + rows,
                                                         2 - du:2 - du + W]
                                            nc.tensor.matmul(
                                                ps[:],
                                                wt[:, idx * 128:(idx + 1) * 128],
                                                rhs,
                                                start=(j == 0), stop=(j == nmm - 1))
                                            j += 1
                                if rb < N_RB:
                                    dst = zt[:].rearrange(
                                        "p (r s) (q t) -> p s t r q",
                                        s=2, t=2)[:, s, t]
                                    src = ps[:].rearrange("p (r q) -> p r q", r=rows)
                                else:
                                    dst = zt[:].rearrange(
                                        "p r (q t) -> p t r q", t=2)[:, t, 0:1]
                                    src = ps[:].rearrange("p (r q) -> p r q", r=1)
                                nc.scalar.activation(dst, src, Copy)

                    def dve_block(ob):
                        # out rows [16ob, 16ob+16); needs zz rows [16ob-1, 16ob+18)
                        t1 = tpool.tile([128, 18, 2 * W], f32, tag="t1")
                        prev = zrow[:] if ob == 0 else zz[ob - 1][:, 15:16]
                        nc.vector.tensor_tensor(
                            t1[:, 0:1], zz[ob][:, 0:1], prev, AluOp.add)
                        nc.vector.tensor_tensor(
                            t1[:, 1:16], zz[ob][:, 1:16], zz[ob][:, 0:15], AluOp.add)
                        nc.vector.tensor_tensor(
                            t1[:, 16:17], zz[ob + 1][:, 0:1], zz[ob][:, 15:16], AluOp.add)
                        nc.vector.tensor_tensor(
                            t1[:, 17:18], zz[ob + 1][:, 1:2], zz[ob + 1][:, 0:1], AluOp.add)
                        t2 = tpool.tile([128, 17, 2 * W], f32, tag="t2")
                        nc.gpsimd.tensor_tensor(
                            t2[:], t1[:, 1:18], t1[:, 0:17], AluOp.add)
                        yt = ypool.tile([128, 16, 2 * W], f32, tag="yt")
                        nc.vector.tensor_tensor(
                            yt[:], t2[:, 0:16], t2[:, 1:17], AluOp.add)
                        nc.scalar.activation(yt[:], yt[:], Prelu,
                                             bias=bt[:, oa:oa + 1],
                                             scale=ACT_GAIN, alpha=ALPHA)
                        nc.gpsimd.tensor_scalar(
                            yt[:], yt[:], CLAMP, -CLAMP, AluOp.min, AluOp.max)
                        nc.sync.dma_start(
                            out=out_ext[img % n_img_store, oa * 128:(oa + 1) * 128,
                                        16 * ob:16 * ob + 16, :],
                            in_=yt[:])

                    for rb in range(N_RB + 1):
                        pe_tile(rb)
                        if rb >= 1:
                            dve_block(rb - 1)
    nc.compile()
    return nc


def _get_nc(n_img: int, n_occ: int):
    key = (n_img, n_occ)
    if key not in _CACHE:
        _CACHE[key] = _build_nc(n_img, n_occ)
    return _CACHE[key]


def kernel(x: np.ndarray, weight: np.ndarray, bias: np.ndarray) -> np.ndarray:
    from concourse.bass_utils import run_bass_kernel_spmd

    x = np.asarray(x, np.float32)
    weight = np.asarray(weight, np.float32)
    bias = np.asarray(bias, np.float32)

    wh = _prep_wh(weight)
    bg = np.ascontiguousarray(
        (bias.astype(np.float64) * ACT_GAIN).astype(np.float32)
        .reshape(N_OCC, 128).T)

    n_total = x.shape[0]
    xq = x.reshape(n_total, N_ICC, 128, H, W)
    xpad = np.zeros((n_total, N_ICC, 128, H + 2, W + 2), np.float32)
    xpad[:, :, :, 1:H + 1, 1:W + 1] = xq

    nc = _get_nc(IMG_PER_CORE, N_OCC)
    in_maps = []
    for c in range(N_CORES):
        sl = np.ascontiguousarray(xpad[c * IMG_PER_CORE:(c + 1) * IMG_PER_CORE])
        in_maps.append({"xp": sl, "wh": wh, "bg": bg})
    res = run_bass_kernel_spmd(nc, in_maps, list(range(N_CORES)))
    out = np.concatenate([res.results[c]["out"] for c in range(N_CORES)], axis=0)
    return out



# revision 13
# speedup vs baseline: 2.9691x; 2.9691x over previous
"""Trainium2 Bass kernel for StyleGAN2-style upsampling ConvLayer.

Reference computation (per image):
  zz = conv_transpose2d(x, (w*WSCALE), stride=2)      # 512ch 64x64 -> 256ch 129x129
  y  = upfirdn2d(zz, fir([1,3,3,1]), pad=1, gain=4)   # 4x4 blur   -> 128x128
  y  = clamp(lrelu(y + bias, 0.2) * sqrt(2), +-256)

This implementation keeps the FIR *out* of the tensor engine (the previous
version folded the horizontal FIR into the weights, doubling PE work):

  - PE computes the bare polyphase conv_transpose zz (129x129) in bf16:
    4 parity groups per 16-row block, psum tiles [128, 8, 64], N=512
    matmuls at 1 cyc/row.  Edge strips (row 128 / col 128) are packed
    into one extra psum pass per unit.
  - ACT evacuates PSUM -> SBUF with bf16 downcast and column
    de-interleave into a padded zz plane (row pitch 132 keeps every
    row-shifted view 4B-aligned).
  - DVE applies the vertical FIR [1,3,3,1] as three box passes in bf16
    2x mode, then h1 and part of h3 at 1x.
  - GPSIMD does h2 and the rest of h3.
  - ACT applies Prelu with the FIR normalization (1/16) and lrelu gain
    folded into scale/bias.  Output is bf16; the host upcasts to fp32
    and applies the (numerically inert) +-256 clamp exactly.

Sharding: data parallel, 2 images per core across 8 NeuronCores.
"""

import numpy as np
import ml_dtypes

N_CORES = 8
IMG_PER_CORE = 2
IN_CH, OUT_CH, K, UP = 512, 256, 3, 2
H = W = 64
WSCALE = float(1.0 / np.sqrt(K * K * IN_CH))
ACT_GAIN = float(np.sqrt(2.0))
CLAMP = 256.0
ALPHA = 0.2
N_ICC = IN_CH // 128   # 4 ic chunks
N_OCC = OUT_CH // 128  # 2 oc chunks
N_B = 8                # main blocks of 16 zz rows (rows 0..127)
PW = 132               # padded row width (even -> 264B pitch, keeps bf16 2x)
RING = 32              # v2 ring rows
H_DVE_FRAC = 8.0 / 16.0  # fraction of h-chain rows on DVE (rest gpsimd)

_CACHE = {}
_ABLATE = set()  # debug: subsets of {"stages", "evac", "finish", "edges"}


def _prep_inputs(x, weight, bias):
    bf = ml_dtypes.bfloat16
    n = x.shape[0]
    xq = x.reshape(n, N_ICC, 128, H, W)
    xpad = np.zeros((n, N_ICC, 128, H + 2, W + 2), bf)
    xpad[:, :, :, 1:H + 1, 1:W + 1] = xq.astype(bf)
    # wt[i, (oa, icc, a, b), o]
    w = (weight.astype(np.float64) * WSCALE).astype(np.float32)
    w = w.reshape(N_OCC, 128, N_ICC, 128, K, K)          # [oa, o, icc, i, a, b]
    wt = np.ascontiguousarray(
        w.transpose(3, 0, 2, 4, 5, 1).astype(bf)         # [i, oa, icc, a, b, o]
    ).reshape(128, N_OCC * N_ICC * K * K * 128)
    bg = np.ascontiguousarray(
        (bias.astype(np.float64) * ACT_GAIN).astype(np.float32)
        .reshape(N_OCC, 128).T)
    return xpad, wt, bg


def _build_nc(n_img: int = IMG_PER_CORE, n_rep: int = 1):
    import concourse.bacc as bacc
    import concourse.mybir as mybir
    import concourse.tile as tile

    f32 = mybir.dt.float32
    bf16 = mybir.dt.bfloat16
    Prelu = mybir.ActivationFunctionType.Prelu
    Copy = mybir.ActivationFunctionType.Copy
    Add = mybir.AluOpType.add

    nc = bacc.Bacc()
    xq_ext = nc.declare_dram_parameter(
        "xq", [n_img, N_ICC, 128, H + 2, W + 2], bf16, isOutput=False)
    wt_ext = nc.declare_dram_parameter(
        "wt", [128, N_OCC * N_ICC * K * K * 128], bf16, isOutput=False)
    bg_ext = nc.declare_dram_parameter("bg", [128, N_OCC], f32, isOutput=False)
    out_ext = nc.declare_dram_parameter(
        "out", [n_img, OUT_CH, 2 * H, 2 * W], bf16, isOutput=True)

    def widx(oa, icc, a, b):
        return ((oa * N_ICC + icc) * K + a) * K + b

    with tile.TileContext(nc) as tc:
        with (
            tc.tile_pool(name="cpool", bufs=1) as cpool,
            tc.tile_pool(name="xpool", bufs=2) as xpool,
            tc.tile_pool(name="plane", bufs=1) as plane,
            tc.tile_pool(name="bpool", bufs=2) as bpool,
            tc.tile_pool(name="ppool", bufs=2, space="PSUM") as ppool,
        ):
            wt = cpool.tile([128, N_OCC * N_ICC * K * K * 128], bf16)
            nc.sync.dma_start(out=wt[:], in_=wt_ext[:])
            bg = cpool.tile([128, N_OCC], f32)
            nc.sync.dma_start(out=bg[:], in_=bg_ext[:])

            # persistent planes; sub-range deps give cross-unit pipelining
            zzP = plane.tile([128, PW, PW], bf16)    # zz row r -> slot r+1
            v1P = plane.tile([128, 130, PW], bf16)   # v1[r]=zz[r]+zz[r+1], slot r+1
            v2R = plane.tile([128, RING, PW], bf16)  # v2[r]=v1[r]+v1[r+1], slot (r+1)%RING
            # interleaved view of zzP: [p, rowpar, colpar, r, c]
            # row slot 2r+i, col slot 2c+j
            zzV = zzP[:].rearrange("p (r i) (c j) -> p i j r c", i=2, j=2)

            def lhs(oa, icc, a, b):
                i = widx(oa, icc, a, b)
                return wt[:, i * 128:(i + 1) * 128]

            def unit(img, oa, xts, out_img):
                # ---- padding memsets (pads stay zero through v passes) ----
                nc.vector.memset(zzP[:, 0:1, :], 0.0)
                nc.vector.memset(zzP[:, 130:132, :], 0.0)
                nc.vector.memset(zzP[:, 1:130, 0:1], 0.0)
                nc.vector.memset(zzP[:, 1:130, 130:132], 0.0)

                # ---- edge strips: col 128 (all rows), row 128 (cols 0..127) ----
                pse = ppool.tile([128, 8, 64], f32, tag="pee")
                pe_flat = pse[:].rearrange("p r c -> p (r c)")
                # strip_e: zz[2k,128] k=0..64 -> [0:65]
                j = 0
                for icc in range(N_ICC):
                    for al in (0, 1):
                        for be in (0, 1):
                            nc.tensor.matmul(
                                pe_flat[:, 0:65], lhs(oa, icc, 2 * al, 2 * be),
                                xts[icc][:, 1 - al:66 - al, 65 - be:66 - be],
                                start=(j == 0), stop=(j == 15),
                                skip_group_check=True)
                            j += 1
                # strip_o: zz[2k+1,128] k=0..63 -> [65:129]
                j = 0
                for icc in range(N_ICC):
                    for be in (0, 1):
                        nc.tensor.matmul(
                            pe_flat[:, 65:129], lhs(oa, icc, 1, 2 * be),
                            xts[icc][:, 1:65, 65 - be:66 - be],
                            start=(j == 0), stop=(j == 7),
                            skip_group_check=True)
                        j += 1
                # R_e: zz[128, 2m] m=0..63 -> [129:193]
                j = 0
                for icc in range(N_ICC):
                    for al in (0, 1):
                        for be in (0, 1):
                            nc.tensor.matmul(
                                pe_flat[:, 129:193], lhs(oa, icc, 2 * al, 2 * be),
                                xts[icc][:, 65 - al:66 - al, 1 - be:65 - be],
                                start=(j == 0), stop=(j == 15),
                                skip_group_check=True)
                            j += 1
                # R_o: zz[128, 2m+1] m=0..63 -> [193:257]
                j = 0
                for icc in range(N_ICC):
                    for al in (0, 1):
                        nc.tensor.matmul(
                            pe_flat[:, 193:257], lhs(oa, icc, 2 * al, 1),
                            xts[icc][:, 65 - al:66 - al, 1:65],
                            start=(j == 0), stop=(j == 7),
                            skip_group_check=True)
                        j += 1
                # evac edges: zz[r,c] -> zzV[i=(r%2==0? via slot r+1...)]
                # zz row 2k -> slot 2k+1 (i=1), row 2k+1 -> slot 2k+2 (i=0)
                # zz col 128 -> slot 129 (j=1,c=64); col 2m -> slot 2m+1 (j=1)
                nc.scalar.activation(
                    zzV[:, 1, 1, 0:65, 64:65],
                    pe_flat[:, 0:65].rearrange("p (r c) -> p r c", c=1), Copy)
                nc.scalar.activation(
                    zzV[:, 0, 1, 1:65, 64:65],
                    pe_flat[:, 65:129].rearrange("p (r c) -> p r c", c=1), Copy)
                nc.scalar.activation(
                    zzV[:, 1, 1, 64:65, 0:64],
                    pe_flat[:, 129:193].rearrange("p (r c) -> p r c", r=1), Copy)
                nc.scalar.activation(
                    zzV[:, 1, 0, 64:65, 1:65],
                    pe_flat[:, 193:257].rearrange("p (r c) -> p r c", r=1), Copy)

                def pe_block(Kb):
                    r0 = 8 * Kb
                    # ee: zz rows 16Kb+2k, cols 2m
                    ps_ee = ppool.tile([128, 8, 64], f32, tag="pee")
                    j = 0
                    for icc in range(N_ICC):
                        for al in (0, 1):
                            for be in (0, 1):
                                nc.tensor.matmul(
                                    ps_ee[:], lhs(oa, icc, 2 * al, 2 * be),
                                    xts[icc][:, r0 + 1 - al:r0 + 9 - al,
                                             1 - be:65 - be],
                                    start=(j == 0), stop=(j == 15))
                                j += 1
                    # eo: rows even, cols odd (b=1)
                    ps_eo = ppool.tile([128, 8, 64], f32, tag="peo")
                    j = 0
                    for icc in range(N_ICC):
                        for al in (0, 1):
                            nc.tensor.matmul(
                                ps_eo[:], lhs(oa, icc, 2 * al, 1),
                                xts[icc][:, r0 + 1 - al:r0 + 9 - al, 1:65],
                                start=(j == 0), stop=(j == 7))
                            j += 1
                    # oe: rows odd (a=1), cols even
                    ps_oe = ppool.tile([128, 8, 64], f32, tag="poe")
                    j = 0
                    for icc in range(N_ICC):
                        for be in (0, 1):
                            nc.tensor.matmul(
                                ps_oe[:], lhs(oa, icc, 1, 2 * be),
                                xts[icc][:, r0 + 1:r0 + 9, 1 - be:65 - be],
                                start=(j == 0), stop=(j == 7))
                            j += 1
                    # oo: rows odd, cols odd
                    ps_oo = ppool.tile([128, 8, 64], f32, tag="poo")
                    j = 0
                    for icc in range(N_ICC):
                        nc.tensor.matmul(
                            ps_oo[:], lhs(oa, icc, 1, 1),
                            xts[icc][:, r0 + 1:r0 + 9, 1:65],
                            start=(j == 0), stop=(j == 3))
                        j += 1
                    # evac: row 16Kb+2k -> slot ..+1 (i=1, r=8Kb+k);
                    #       row 16Kb+2k+1 -> slot ..+2 (i=0, r=8Kb+k+1)
                    # col 2m -> slot 2m+1 (j=1, c=m); col 2m+1 -> slot 2m+2 (j=0, c=m+1)
                    if "evac" in _ABLATE:
                        return
                    nc.scalar.activation(zzV[:, 1, 1, r0:r0 + 8, 0:64], ps_ee[:], Copy)
                    nc.scalar.activation(zzV[:, 1, 0, r0:r0 + 8, 1:65], ps_eo[:], Copy)
                    nc.scalar.activation(zzV[:, 0, 1, r0 + 1:r0 + 9, 0:64], ps_oe[:], Copy)
                    nc.scalar.activation(zzV[:, 0, 0, r0 + 1:r0 + 9, 1:65], ps_oo[:], Copy)

                ytiles = {}

                def vh_stages(Kb):
                    # v1 window: rows [16K-2, 16K+14) of domain [-1, 129)
                    a0, a1 = max(16 * Kb - 2, -1), min(16 * Kb + 14, 129)
                    if a0 < a1:
                        nc.vector.tensor_tensor(
                            v1P[:, a0 + 1:a1 + 1, :],
                            zzP[:, a0 + 1:a1 + 1, :],
                            zzP[:, a0 + 2:a1 + 2, :], Add)
                    # v2 window: rows [16K-4, 16K+12) of domain [-1, 128)
                    b0, b1 = max(16 * Kb - 4, -1), min(16 * Kb + 12, 128)
                    r = b0
                    while r < b1:
                        s = (r + 1) % RING
                        n = min(b1 - r, RING - s)
                        nc.vector.tensor_tensor(
                            v2R[:, s:s + n, :],
                            v1P[:, r + 1:r + 1 + n, :],
                            v1P[:, r + 2:r + 2 + n, :], Add)
                        r += n
                    # out-rows window: [16K-6, 16K+10) of [0, 128)
                    c0, c1 = max(16 * Kb - 6, 0), min(16 * Kb + 10, 128)
                    if c0 >= c1:
                        return
                    nrows = c1 - c0
                    v3 = bpool.tile([128, 16, PW], bf16, tag="v3")
                    # v3[i] = v2[c0+i-1] + v2[c0+i] ; src slots (c0+i)%RING, (c0+i+1)%RING
                    r = c0
                    while r < c1:
                        s0 = r % RING
                        s1 = (r + 1) % RING
                        n = min(c1 - r, RING - s0, RING - s1)
                        nc.vector.tensor_tensor(
                            v3[:, r - c0:r - c0 + n, :],
                            v2R[:, s0:s0 + n, :],
                            v2R[:, s1:s1 + n, :], Add)
                        r += n
                    # h chain split by rows: DVE rows [0:nd), gpsimd rows [nd:nrows)
                    # (h passes are row-independent: no cross-engine waits)
                    h1 = bpool.tile([128, 16, PW], bf16, tag="h1")
                    h2 = bpool.tile([128, 16, PW], bf16, tag="h2")
                    y = bpool.tile([128, 16, 128], bf16, tag="y", bufs=3)
                    nd = min(int(round(H_DVE_FRAC * 16)), nrows)
                    if nd > 0:
                        nc.vector.tensor_tensor(
                            h1[:, 0:nd, 0:130],
                            v3[:, 0:nd, 0:130], v3[:, 0:nd, 1:131], Add)
                        nc.vector.tensor_tensor(
                            h2[:, 0:nd, 0:129],
                            h1[:, 0:nd, 0:129], h1[:, 0:nd, 1:130], Add)
                        nc.vector.tensor_tensor(
                            y[:, 0:nd, :],
                            h2[:, 0:nd, 0:128], h2[:, 0:nd, 1:129], Add)
                    if nd < nrows:
                        nc.gpsimd.tensor_tensor(
                            h1[:, nd:nrows, 0:130],
                            v3[:, nd:nrows, 0:130], v3[:, nd:nrows, 1:131], Add)
                        nc.gpsimd.tensor_tensor(
                            h2[:, nd:nrows, 0:129],
                            h1[:, nd:nrows, 0:129], h1[:, nd:nrows, 1:130], Add)
                        nc.gpsimd.tensor_tensor(
                            y[:, nd:nrows, :],
                            h2[:, nd:nrows, 0:128], h2[:, nd:nrows, 1:129], Add)
                    ytiles[Kb] = (y, c0, c1)

                def finish(Kb):
                    if Kb not in ytiles:
                        return
                    y, c0, c1 = ytiles.pop(Kb)
                    nrows = c1 - c0
                    if "prelu" not in _ABLATE:
                        func = (mybir.ActivationFunctionType.Identity
                                if "identity" in _ABLATE else Prelu)
                        nc.scalar.activation(
                            y[:, 0:nrows, :], y[:, 0:nrows, :], func,
                            bias=bg[:, oa:oa + 1],
                            scale=ACT_GAIN / 16.0, alpha=ALPHA)
                    # out stores ride the ACT HWDGE ring so x prefetch on the
                    # SP ring is never queued behind them
                    if "outdma" not in _ABLATE:
                        nc.scalar.dma_start(
                            out=out_ext[out_img, oa * 128:(oa + 1) * 128, c0:c1, :],
                            in_=y[:, 0:nrows, :])

                skip_stages = "stages" in _ABLATE
                skip_finish = "finish" in _ABLATE or skip_stages
                for Kb in range(N_B):
                    pe_block(Kb)
                    if not skip_stages and Kb >= 1:
                        vh_stages(Kb - 1)
                    if not skip_finish and Kb >= 2:
                        finish(Kb - 2)
                if not skip_stages:
                    vh_stages(N_B - 1)
                    vh_stages(N_B)
                if not skip_finish:
                    finish(N_B - 2)
                    finish(N_B - 1)
                    finish(N_B)

            for it in range(n_img * n_rep):
                img = it % n_img
                xts = []
                for icc in range(N_ICC):
                    xt = xpool.tile([128, H + 2, W + 2], bf16, tag=f"x{icc}")
                    nc.sync.dma_start(out=xt[:], in_=xq_ext[img, icc])
                    xts.append(xt)
                for oa in range(N_OCC):
                    unit(img, oa, xts, img)
    nc.compile()
    return nc


def _get_nc(n_img: int = IMG_PER_CORE, n_rep: int = 1):
    key = (n_img, n_rep)
    if key not in _CACHE:
        _CACHE[key] = _build_nc(n_img, n_rep)
    return _CACHE[key]


def kernel(x: np.ndarray, weight: np.ndarray, bias: np.ndarray) -> np.ndarray:
    from concourse.bass_utils import run_bass_kernel_spmd

    x = np.asarray(x, np.float32)
    weight = np.asarray(weight, np.float32)
    bias = np.asarray(bias, np.float32)

    xpad, wt, bg = _prep_inputs(x, weight, bias)

    nc = _get_nc()
    in_maps = []
    for c in range(N_CORES):
        sl = np.ascontiguousarray(xpad[c * IMG_PER_CORE:(c + 1) * IMG_PER_CORE])
        in_maps.append({"xq": sl, "wt": wt, "bg": bg})
    res = run_bass_kernel_spmd(nc, in_maps, list(range(N_CORES)))
    out = np.concatenate([res.results[c]["out"] for c in range(N_CORES)], axis=0)
    out = out.astype(np.float32)
    np.clip(out, -CLAMP, CLAMP, out=out)
    return out


# revision 14
# speedup vs baseline: 4.0233x; 1.3551x over previous
"""Trainium2 Bass kernel for StyleGAN2-style upsampling ConvLayer.

Reference computation (per image):
  zz = conv_transpose2d(x, (w*WSCALE), stride=2)      # 512ch 64x64 -> 256ch 129x129
  y  = upfirdn2d(zz, fir([1,3,3,1]), pad=1, gain=4)   # 4x4 blur   -> 128x128
  y  = clamp(lrelu(y + bias, 0.2) * sqrt(2), +-256)

This implementation keeps the FIR *out* of the tensor engine (the previous
version folded the horizontal FIR into the weights, doubling PE work):

  - PE computes the bare polyphase conv_transpose zz (129x129) in bf16:
    4 parity groups per 16-row block, psum tiles [128, 8, 64], N=512
    matmuls at 1 cyc/row.  Edge strips (row 128 / col 128) are packed
    into one extra psum pass per unit.
  - ACT evacuates PSUM -> SBUF with bf16 downcast and column
    de-interleave into a padded zz plane (row pitch 132 keeps every
    row-shifted view 4B-aligned).
  - DVE applies the vertical FIR [1,3,3,1] as three box passes in bf16
    2x mode, then h1 and part of h3 at 1x.
  - GPSIMD does h2 and the rest of h3.
  - ACT applies Prelu with the FIR normalization (1/16) and lrelu gain
    folded into scale/bias.  Output is bf16; the host upcasts to fp32
    and applies the (numerically inert) +-256 clamp exactly.

Sharding: data parallel, 2 images per core across 8 NeuronCores.
"""

import numpy as np
import ml_dtypes

N_CORES = 8
IMG_PER_CORE = 2
IN_CH, OUT_CH, K, UP = 512, 256, 3, 2
H = W = 64
WSCALE = float(1.0 / np.sqrt(K * K * IN_CH))
ACT_GAIN = float(np.sqrt(2.0))
CLAMP = 256.0
ALPHA = 0.2
N_ICC = IN_CH // 128   # 4 ic chunks
N_OCC = OUT_CH // 128  # 2 oc chunks
N_B = 8                # main blocks of 16 zz rows (rows 0..127)
PW = 132               # padded row width (even -> 264B pitch, keeps bf16 2x)
RING = 32              # v2 ring rows
H_DVE_FRAC = 8.0 / 16.0  # fraction of h-chain rows on DVE (rest gpsimd)

_CACHE = {}
_ABLATE = set()  # debug: subsets of {"stages", "evac", "finish", "edges"}


def _prep_inputs(x, weight, bias):
    bf = ml_dtypes.bfloat16
    n = x.shape[0]
    xq = x.reshape(n, N_ICC, 128, H, W)
    xpad = np.zeros((n, N_ICC, 128, H + 2, W + 2), bf)
    xpad[:, :, :, 1:H + 1, 1:W + 1] = xq.astype(bf)
    # wt[i, (oa, icc, a, b), o]
    w = (weight.astype(np.float64) * WSCALE).astype(np.float32)
    w = w.reshape(N_OCC, 128, N_ICC, 128, K, K)          # [oa, o, icc, i, a, b]
    wt = np.ascontiguousarray(
        w.transpose(3, 0, 2, 4, 5, 1).astype(bf)         # [i, oa, icc, a, b, o]
    ).reshape(128, N_OCC * N_ICC * K * K * 128)
    bg = np.ascontiguousarray(
        (bias.astype(np.float64) * ACT_GAIN).astype(np.float32)
        .reshape(N_OCC, 128).T)
    return xpad, wt, bg


def _build_nc(n_img: int = IMG_PER_CORE, n_rep: int = 1):
    import concourse.bacc as bacc
    import concourse.mybir as mybir
    import concourse.tile as tile

    f32 = mybir.dt.float32
    bf16 = mybir.dt.bfloat16
    Prelu = mybir.ActivationFunctionType.Prelu
    Copy = mybir.ActivationFunctionType.Copy
    Add = mybir.AluOpType.add

    nc = bacc.Bacc()
    xq_ext = nc.declare_dram_parameter(
        "xq", [n_img, N_ICC, 128, H + 2, W + 2], bf16, isOutput=False)
    wt_ext = nc.declare_dram_parameter(
        "wt", [128, N_OCC * N_ICC * K * K * 128], bf16, isOutput=False)
    bg_ext = nc.declare_dram_parameter("bg", [128, N_OCC], f32, isOutput=False)
    out_ext = nc.declare_dram_parameter(
        "out", [n_img, OUT_CH, 2 * H, 2 * W], bf16, isOutput=True)

    def widx(oa, icc, a, b):
        return ((oa * N_ICC + icc) * K + a) * K + b

    with tile.TileContext(nc) as tc:
        with (
            tc.tile_pool(name="cpool", bufs=1) as cpool,
            tc.tile_pool(name="xpool", bufs=2) as xpool,
            tc.tile_pool(name="plane", bufs=1) as plane,
            tc.tile_pool(name="bpool", bufs=2) as bpool,
            tc.tile_pool(name="ppool", bufs=2, space="PSUM") as ppool,
        ):
            # weights ride the ACT ring in parallel with x loads on SP
            wt = cpool.tile([128, N_OCC * N_ICC * K * K * 128], bf16)
            nc.scalar.dma_start(out=wt[:], in_=wt_ext[:])
            bg = cpool.tile([128, N_OCC], f32)
            nc.scalar.dma_start(out=bg[:], in_=bg_ext[:])

            # persistent planes; sub-range deps give cross-unit pipelining
            zzP = plane.tile([128, PW, PW], bf16)    # zz row r -> slot r+1
            v1P = plane.tile([128, 130, PW], bf16)   # v1[r]=zz[r]+zz[r+1], slot r+1
            v2R = plane.tile([128, RING, PW], bf16)  # v2[r]=v1[r]+v1[r+1], slot (r+1)%RING
            # interleaved view of zzP: [p, rowpar, colpar, r, c]
            # row slot 2r+i, col slot 2c+j
            zzV = zzP[:].rearrange("p (r i) (c j) -> p i j r c", i=2, j=2)

            def lhs(oa, icc, a, b):
                i = widx(oa, icc, a, b)
                return wt[:, i * 128:(i + 1) * 128]

            def unit(img, oa, xts, out_img):
                # ---- padding memsets (pads stay zero through v passes) ----
                nc.vector.memset(zzP[:, 0:1, :], 0.0)
                nc.vector.memset(zzP[:, 130:132, :], 0.0)
                nc.vector.memset(zzP[:, 1:130, 0:1], 0.0)
                nc.vector.memset(zzP[:, 1:130, 130:132], 0.0)

                # ---- edge strips: col 128 (all rows), row 128 (cols 0..127) ----
                pse = ppool.tile([128, 8, 64], f32, tag="pee")
                pe_flat = pse[:].rearrange("p r c -> p (r c)")
                # strip_e: zz[2k,128] k=0..64 -> [0:65]
                j = 0
                for icc in range(N_ICC):
                    for al in (0, 1):
                        for be in (0, 1):
                            nc.tensor.matmul(
                                pe_flat[:, 0:65], lhs(oa, icc, 2 * al, 2 * be),
                                xts[icc][:, 1 - al:66 - al, 65 - be:66 - be],
                                start=(j == 0), stop=(j == 15),
                                skip_group_check=True)
                            j += 1
                # strip_o: zz[2k+1,128] k=0..63 -> [65:129]
                j = 0
                for icc in range(N_ICC):
                    for be in (0, 1):
                        nc.tensor.matmul(
                            pe_flat[:, 65:129], lhs(oa, icc, 1, 2 * be),
                            xts[icc][:, 1:65, 65 - be:66 - be],
                            start=(j == 0), stop=(j == 7),
                            skip_group_check=True)
                        j += 1
                # R_e: zz[128, 2m] m=0..63 -> [129:193]
                j = 0
                for icc in range(N_ICC):
                    for al in (0, 1):
                        for be in (0, 1):
                            nc.tensor.matmul(
                                pe_flat[:, 129:193], lhs(oa, icc, 2 * al, 2 * be),
                                xts[icc][:, 65 - al:66 - al, 1 - be:65 - be],
                                start=(j == 0), stop=(j == 15),
                                skip_group_check=True)
                            j += 1
                # R_o: zz[128, 2m+1] m=0..63 -> [193:257]
                j = 0
                for icc in range(N_ICC):
                    for al in (0, 1):
                        nc.tensor.matmul(
                            pe_flat[:, 193:257], lhs(oa, icc, 2 * al, 1),
                            xts[icc][:, 65 - al:66 - al, 1:65],
                            start=(j == 0), stop=(j == 7),
                            skip_group_check=True)
                        j += 1
                # evac edges: zz[r,c] -> zzV[i=(r%2==0? via slot r+1...)]
                # zz row 2k -> slot 2k+1 (i=1), row 2k+1 -> slot 2k+2 (i=0)
                # zz col 128 -> slot 129 (j=1,c=64); col 2m -> slot 2m+1 (j=1)
                nc.scalar.activation(
                    zzV[:, 1, 1, 0:65, 64:65],
                    pe_flat[:, 0:65].rearrange("p (r c) -> p r c", c=1), Copy)
                nc.scalar.activation(
                    zzV[:, 0, 1, 1:65, 64:65],
                    pe_flat[:, 65:129].rearrange("p (r c) -> p r c", c=1), Copy)
                nc.scalar.activation(
                    zzV[:, 1, 1, 64:65, 0:64],
                    pe_flat[:, 129:193].rearrange("p (r c) -> p r c", r=1), Copy)
                nc.scalar.activation(
                    zzV[:, 1, 0, 64:65, 1:65],
                    pe_flat[:, 193:257].rearrange("p (r c) -> p r c", r=1), Copy)

                def pe_block(Kb):
                    r0 = 8 * Kb
                    # ee: zz rows 16Kb+2k, cols 2m
                    ps_ee = ppool.tile([128, 8, 64], f32, tag="pee")
                    j = 0
                    for icc in range(N_ICC):
                        for al in (0, 1):
                            for be in (0, 1):
                                nc.tensor.matmul(
                                    ps_ee[:], lhs(oa, icc, 2 * al, 2 * be),
                                    xts[icc][:, r0 + 1 - al:r0 + 9 - al,
                                             1 - be:65 - be],
                                    start=(j == 0), stop=(j == 15))
                                j += 1
                    # eo: rows even, cols odd (b=1)
                    ps_eo = ppool.tile([128, 8, 64], f32, tag="peo")
                    j = 0
                    for icc in range(N_ICC):
                        for al in (0, 1):
                            nc.tensor.matmul(
                                ps_eo[:], lhs(oa, icc, 2 * al, 1),
                                xts[icc][:, r0 + 1 - al:r0 + 9 - al, 1:65],
                                start=(j == 0), stop=(j == 7))
                            j += 1
                    # oe: rows odd (a=1), cols even
                    ps_oe = ppool.tile([128, 8, 64], f32, tag="poe")
                    j = 0
                    for icc in range(N_ICC):
                        for be in (0, 1):
                            nc.tensor.matmul(
                                ps_oe[:], lhs(oa, icc, 1, 2 * be),
                                xts[icc][:, r0 + 1:r0 + 9, 1 - be:65 - be],
                                start=(j == 0), stop=(j == 7))
                            j += 1
                    # oo: rows odd, cols odd
                    ps_oo = ppool.tile([128, 8, 64], f32, tag="poo")
                    j = 0
                    for icc in range(N_ICC):
                        nc.tensor.matmul(
                            ps_oo[:], lhs(oa, icc, 1, 1),
                            xts[icc][:, r0 + 1:r0 + 9, 1:65],
                            start=(j == 0), stop=(j == 3))
                        j += 1
                    # evac: row 16Kb+2k -> slot ..+1 (i=1, r=8Kb+k);
                    #       row 16Kb+2k+1 -> slot ..+2 (i=0, r=8Kb+k+1)
                    # col 2m -> slot 2m+1 (j=1, c=m); col 2m+1 -> slot 2m+2 (j=0, c=m+1)
                    if "evac" in _ABLATE:
                        return
                    nc.scalar.activation(zzV[:, 1, 1, r0:r0 + 8, 0:64], ps_ee[:], Copy)
                    nc.scalar.activation(zzV[:, 1, 0, r0:r0 + 8, 1:65], ps_eo[:], Copy)
                    nc.scalar.activation(zzV[:, 0, 1, r0 + 1:r0 + 9, 0:64], ps_oe[:], Copy)
                    nc.scalar.activation(zzV[:, 0, 0, r0 + 1:r0 + 9, 1:65], ps_oo[:], Copy)

                ytiles = {}

                def vh_stages(Kb):
                    # v1 window: rows [16K-2, 16K+14) of domain [-1, 129)
                    a0, a1 = max(16 * Kb - 2, -1), min(16 * Kb + 14, 129)
                    if a0 < a1:
                        nc.vector.tensor_tensor(
                            v1P[:, a0 + 1:a1 + 1, :],
                            zzP[:, a0 + 1:a1 + 1, :],
                            zzP[:, a0 + 2:a1 + 2, :], Add)
                    # v2 window: rows [16K-4, 16K+12) of domain [-1, 128)
                    b0, b1 = max(16 * Kb - 4, -1), min(16 * Kb + 12, 128)
                    r = b0
                    while r < b1:
                        s = (r + 1) % RING
                        n = min(b1 - r, RING - s)
                        nc.vector.tensor_tensor(
                            v2R[:, s:s + n, :],
                            v1P[:, r + 1:r + 1 + n, :],
                            v1P[:, r + 2:r + 2 + n, :], Add)
                        r += n
                    # out-rows window: [16K-6, 16K+10) of [0, 128)
                    c0, c1 = max(16 * Kb - 6, 0), min(16 * Kb + 10, 128)
                    if c0 >= c1:
                        return
                    nrows = c1 - c0
                    v3 = bpool.tile([128, 16, PW], bf16, tag="v3")
                    # v3[i] = v2[c0+i-1] + v2[c0+i] ; src slots (c0+i)%RING, (c0+i+1)%RING
                    r = c0
                    while r < c1:
                        s0 = r % RING
                        s1 = (r + 1) % RING
                        n = min(c1 - r, RING - s0, RING - s1)
                        nc.vector.tensor_tensor(
                            v3[:, r - c0:r - c0 + n, :],
                            v2R[:, s0:s0 + n, :],
                            v2R[:, s1:s1 + n, :], Add)
                        r += n
                    # h chain split by rows: DVE rows [0:nd), gpsimd rows [nd:nrows)
                    # (h passes are row-independent: no cross-engine waits)
                    h1 = bpool.tile([128, 16, PW], bf16, tag="h1")
                    h2 = bpool.tile([128, 16, PW], bf16, tag="h2")
                    y = bpool.tile([128, 16, 128], bf16, tag="y", bufs=3)
                    nd = min(int(round(H_DVE_FRAC * 16)), nrows)
                    if nd > 0:
                        nc.vector.tensor_tensor(
                            h1[:, 0:nd, 0:130],
                            v3[:, 0:nd, 0:130], v3[:, 0:nd, 1:131], Add)
                        nc.vector.tensor_tensor(
                            h2[:, 0:nd, 0:129],
                            h1[:, 0:nd, 0:129], h1[:, 0:nd, 1:130], Add)
                        nc.vector.tensor_tensor(
                            y[:, 0:nd, :],
                            h2[:, 0:nd, 0:128], h2[:, 0:nd, 1:129], Add)
                    if nd < nrows:
                        nc.gpsimd.tensor_tensor(
                            h1[:, nd:nrows, 0:130],
                            v3[:, nd:nrows, 0:130], v3[:, nd:nrows, 1:131], Add)
                        nc.gpsimd.tensor_tensor(
                            h2[:, nd:nrows, 0:129],
                            h1[:, nd:nrows, 0:129], h1[:, nd:nrows, 1:130], Add)
                        nc.gpsimd.tensor_tensor(
                            y[:, nd:nrows, :],
                            h2[:, nd:nrows, 0:128], h2[:, nd:nrows, 1:129], Add)
                    ytiles[Kb] = (y, c0, c1)

                def finish(Kb):
                    if Kb not in ytiles:
                        return
                    y, c0, c1 = ytiles.pop(Kb)
                    nrows = c1 - c0
                    if "prelu" not in _ABLATE:
                        func = (mybir.ActivationFunctionType.Identity
                                if "identity" in _ABLATE else Prelu)
                        nc.scalar.activation(
                            y[:, 0:nrows, :], y[:, 0:nrows, :], func,
                            bias=bg[:, oa:oa + 1],
                            scale=ACT_GAIN / 16.0, alpha=ALPHA)
                    # out stores ride the ACT HWDGE ring so x prefetch on the
                    # SP ring is never queued behind them
                    if "outdma" not in _ABLATE:
                        nc.scalar.dma_start(
                            out=out_ext[out_img, oa * 128:(oa + 1) * 128, c0:c1, :],
                            in_=y[:, 0:nrows, :])

                skip_stages = "stages" in _ABLATE
                skip_finish = "finish" in _ABLATE or skip_stages
                for Kb in range(N_B):
                    pe_block(Kb)
                    if not skip_stages and Kb >= 1:
                        vh_stages(Kb - 1)
                    if not skip_finish and Kb >= 2:
                        finish(Kb - 2)
                if not skip_stages:
                    vh_stages(N_B - 1)
                    vh_stages(N_B)
                if not skip_finish:
                    finish(N_B - 2)
                    finish(N_B - 1)
                    finish(N_B)

            for it in range(n_img * n_rep):
                img = it % n_img
                xts = []
                for icc in range(N_ICC):
                    xt = xpool.tile([128, H + 2, W + 2], bf16, tag=f"x{icc}")
                    nc.sync.dma_start(out=xt[:], in_=xq_ext[img, icc])
                    xts.append(xt)
                for oa in range(N_OCC):
                    unit(img, oa, xts, img)
    nc.compile()
    return nc


def _get_nc(n_img: int = IMG_PER_CORE, n_rep: int = 1):
    key = (n_img, n_rep)
    if key not in _CACHE:
        _CACHE[key] = _build_nc(n_img, n_rep)
    return _CACHE[key]


def kernel(x: np.ndarray, weight: np.ndarray, bias: np.ndarray) -> np.ndarray:
    from concourse.bass_utils import run_bass_kernel_spmd

    x = np.asarray(x, np.float32)
    weight = np.asarray(weight, np.float32)
    bias = np.asarray(bias, np.float32)

    xpad, wt, bg = _prep_inputs(x, weight, bias)

    nc = _get_nc()
    in_maps = []
    for c in range(N_CORES):
        sl = np.ascontiguousarray(xpad[c * IMG_PER_CORE:(c + 1) * IMG_PER_CORE])
        in_maps.append({"xq": sl, "wt": wt, "bg": bg})
    res = run_bass_kernel_spmd(nc, in_maps, list(range(N_CORES)))
    out = np.concatenate([res.results[c]["out"] for c in range(N_CORES)], axis=0)
    out = out.astype(np.float32)
    np.clip(out, -CLAMP, CLAMP, out=out)
    return out
